# revision 12
# baseline (speedup 1.0000x reference)
"""2D Haar DWT (pywt 'haar' dwt2) on 8 Trainium2 NeuronCores via Bass/Tile.

Input:  x [16, 64, 256, 256] f32
Output: (LL, LH, HL, HH), each [16, 64, 128, 128] f32, matching
        LL = (a+b+c+d)/2 etc. per 2x2 block [[a, b], [c, d]].

Sharding: batch dim 16 -> 2 per core across 8 cores, no communication.

Per-core plan: 128 images in groups of 4-8 (small edge groups shorten
pipeline fill/drain). Per group: one contiguous 1-2 MB HWDGE load brings
[128 pair-rows, imgs, (top_row|bot_row)] into SBUF with 2 KB descriptors;
ACT pre-halves the bottom rows; DVE does the stride-2 column butterfly
(4 tensor_tensor) then the row combine with the x0.5 folded in via
scalar_tensor_tensor (4 ops); one store per group writes all 4 quadrants
row-interleaved to o4[b,c,k,q,w] so store descriptors are 2 KB as well
(the host de-interleaves q afterwards - free). Loads issue on the scalar
HWDGE ring, stores on the sync ring (last groups on scalar to drain the
tail in parallel), so store sem-waits never block load triggers.

Measured on trn2: ~208 us/core vs the 187 us HBM roofline (67.1 MB at
358 GB/s); HBM utilization is ~1.0-1.15 wall-to-wall, the remaining
~18 us being fixed NEFF preamble + Tile exit barrier.
"""

from contextlib import ExitStack

import numpy as np

SHARD_B, C, H, W = 2, 64, 256, 256
IMGS = SHARD_B * C          # 128 images per core
HP, WH = H // 2, W // 2
GROUP_IMGS = 8
N_CORES = 8
OUT_NAMES = ("ll", "lh", "hl", "hh")


def _build_nc(bufs: int = 3, group_imgs: int = GROUP_IMGS):
    import concourse.bacc as bacc
    import concourse.mybir as mybir
    import concourse.tile as tile

    nc = bacc.Bacc()
    x = nc.dram_tensor("x", [SHARD_B, C, H, W], mybir.dt.float32, kind="ExternalInput")
    # All 4 quadrants row-interleaved: o4[b, c, k, q, w]; q in (ll, lh, hl, hh).
    # This makes each output DMA descriptor 2 KB instead of 512 B.
    o4 = nc.dram_tensor(
        "o4", [SHARD_B, C, HP, 4, WH], mybir.dt.float32, kind="ExternalOutput"
    )
    xg = x[:, :, :, :].rearrange("b c (hp two) w -> (b c) hp (two w)", two=2)
    o4g = o4[:, :, :, :, :].rearrange("b c k q w -> (b c) k (q w)")

    # Asymmetric grouping: small first/last groups shorten pipeline fill/drain.
    sizes = [2, 2, 4] + [8] * 14 + [4, 2, 2]
    assert sum(sizes) == IMGS
    with tile.TileContext(nc) as tc, ExitStack() as ctx:
        xpool = ctx.enter_context(tc.tile_pool(name="xin", bufs=bufs + 1))
        spool = ctx.enter_context(tc.tile_pool(name="srow", bufs=bufs))
        dpool = ctx.enter_context(tc.tile_pool(name="drow", bufs=bufs))
        opool = ctx.enter_context(tc.tile_pool(name="outs", bufs=bufs + 1))
        j0 = 0
        for g_idx, gi in enumerate(sizes):
            j1 = j0 + gi
            store_eng = nc.scalar if g_idx >= len(sizes) - 4 else nc.sync
            xt = xpool.tile([HP, gi, 2 * W], mybir.dt.float32, tag="xt")
            nc.scalar.dma_start(
                out=xt[:, :, :], in_=xg[j0:j1].rearrange("j p tw -> p j tw")
            )
            bt = spool.tile([HP, gi, W], mybir.dt.float32, tag="bt")
            nc.scalar.mul(bt[:, :, :], xt[:, :, W : 2 * W], 0.5)
            te = xt[:, :, 0:W:2]
            to = xt[:, :, 1:W:2]
            be = bt[:, :, 0:W:2]
            bo = bt[:, :, 1:W:2]
            cst = dpool.tile([HP, gi, WH], mybir.dt.float32, tag="cst")
            cdt = dpool.tile([HP, gi, WH], mybir.dt.float32, tag="cdt")
            csb = dpool.tile([HP, gi, WH], mybir.dt.float32, tag="csb")
            cdb = dpool.tile([HP, gi, WH], mybir.dt.float32, tag="cdb")
            nc.vector.tensor_add(cst[:, :, :], te, to)
            nc.vector.tensor_sub(cdt[:, :, :], te, to)
            nc.vector.tensor_add(csb[:, :, :], be, bo)
            nc.vector.tensor_sub(cdb[:, :, :], be, bo)
            ot = opool.tile([HP, gi, 4, WH], mybir.dt.float32, tag="o4t")
            combos = (
                (0, cst, csb, mybir.AluOpType.add),
                (1, cst, csb, mybir.AluOpType.subtract),
                (2, cdt, cdb, mybir.AluOpType.add),
                (3, cdt, cdb, mybir.AluOpType.subtract),
            )
            for q, tin, bin_, op1 in combos:
                nc.vector.scalar_tensor_tensor(
                    ot[:, :, q, :], tin[:, :, :], 0.5, bin_[:, :, :],
                    mybir.AluOpType.mult, op1,
                )
            store_eng.dma_start(
                out=o4g[j0:j1].rearrange("j k qw -> k j qw"),
                in_=ot[:, :, :, :].rearrange("k j q w -> k j (q w)"),
            )
            j0 = j1
    nc.compile()
    return nc


_NC_CACHE = None


def _get_nc():
    global _NC_CACHE
    if _NC_CACHE is None:
        _NC_CACHE = _build_nc()
    return _NC_CACHE


def run_sharded(x: np.ndarray, trace: bool = False):
    """Run the SPMD kernel; returns (BassKernelResults, outputs dict of full arrays)."""
    from concourse.bass_utils import run_bass_kernel_spmd

    x = np.ascontiguousarray(x, dtype=np.float32)
    nc = _get_nc()
    in_maps = [
        {"x": x[i * SHARD_B : (i + 1) * SHARD_B]} for i in range(N_CORES)
    ]
    br = run_bass_kernel_spmd(nc, in_maps, list(range(N_CORES)), trace=trace)
    o4 = np.concatenate(
        [np.asarray(br.results[i]["o4"]).reshape(SHARD_B, C, HP, 4, WH)
         for i in range(N_CORES)],
        axis=0,
    )
    full = {
        name: np.ascontiguousarray(o4[:, :, :, q, :])
        for q, name in enumerate(OUT_NAMES)
    }
    return br, full


def kernel(x: np.ndarray):
    _, full = run_sharded(x, trace=False)
    return full["ll"], full["lh"], full["hl"], full["hh"]
